# revision 18
# baseline (speedup 1.0000x reference)
"""Trainium2 Bass kernel for nn_Attention_558345749040.

Reference (per batch b, H=8 heads of d=64, S=4096, E=512):
    Q = Q_seq @ WQ ; K = K_seq @ WK ; V = V_seq @ WV
    A = (Q * K) / 8                      (elementwise)
    softmax over each head's 64-wide feature group, positions j >= V_len[b]
    masked out (V_len == 0 degenerates to a uniform 1/64 softmax)
    O = softmax * V, rows s >= Q_len[b] zeroed

Structure exploited (all derived from the runtime Q_len / V_len values, so
the compiled schedule is input-shape-specialized but value-generic):
  * Rows s >= Q_len[b] are zero: only ceil(Q_len/128) 128-token chunks per
    batch carry live data. Live chunks are repartitioned evenly across the
    8 cores (token-balanced data parallel), removing the Q_len imbalance.
  * Only head positions j < V_len[b] matter: the Q/K/V matmuls select the
    8*V_len live weight columns through a strided moving AP over the shared
    full weight tiles (PE matmul cost scales with output free size), the
    softmax runs on vl-wide groups, and only packed columns are stored; the
    host scatters them back into a zero canvas. Full-quota slots need no
    masking at all; remainder chunks share mixed-width slots and get an
    additive -1e4 pre-softmax mask (fused multiply-add, one DVE op).
  * V_len == 0 batches reduce to O = V/64: V-matmul-only slots.
  * fp16 transport + fp16 matmuls throughout (measured rel err 3.6e-3 vs
    the 2e-2 gate; bf16 Q/K fails at 2.5e-2, fp8 V fails at 3.7e-2).
  * HWDGE DMA triggers cost a flat ~625ns on one shared generator: loads
    are batched 4-contraction-chunks-per-trigger, stores one per superslot
    (issued from the ACT queue so input loads never queue behind them).
  * The Q_len row mask rides the softmax-weight multiply as a per-partition
    scalar (fused (e*qm)*r), so V flows from PSUM straight into the final
    elementwise multiply with no staging copy.

Every core runs the same instruction stream (SPMD single-NEFF constraint):
the slot schedule (widths/kinds) is identical across cores; which batch
chunk a slot processes is pure data (gathered inputs + per-slot masks).
"""

import numpy as np
import ml_dtypes

B, S, EMB = 8, 4096, 512
H, D = 8, 64
NCORES = 8
KC = EMB // 128          # 4 contraction chunks
SUP = 8                  # slots per input-DMA superslot

_CACHE = {}


def _plan(Q_len, V_len):
    """Slot schedule shared by all cores + per-core chunk assignment.

    Returns (slots, assign, total_L, mix_L) where slots[j] holds
    {kind: 'reg'|'mix'|'vonly', c, L, off, moff} and assign[i][j] is
    (batch, tok0) for the chunk core i processes in slot j (None = dummy).
    """
    entries = []  # (slotdict, percore list)

    def chunks_of(b):
        ql = int(Q_len[b, 0])
        return -(-ql // 128) if ql > 0 else 0

    rem = []
    for b in range(B):
        nch = chunks_of(b)
        if nch == 0:
            continue
        vl = int(V_len[b, 0])
        if vl == 0:
            quota = -(-nch // NCORES)
            for t in range(quota):
                per = [(b, (t * NCORES + i) * 128)
                       if t * NCORES + i < nch else None
                       for i in range(NCORES)]
                entries.append(({"kind": "vonly", "c": D, "L": 8 * D}, per))
        else:
            fq = nch // NCORES
            for t in range(fq):
                per = [(b, (t * NCORES + i) * 128) for i in range(NCORES)]
                entries.append(({"kind": "reg", "c": vl, "L": 8 * vl}, per))
            for ch in range(NCORES * fq, nch):
                rem.append((vl, b, ch))

    # Remainder chunks: sort by width desc, deal round-robin into mixed
    # slots whose width is the max of their 8 chunks (others get a -1e4
    # additive mask on the padding columns).
    rem.sort(key=lambda x: -x[0])
    for m in range(0, len(rem), NCORES):
        grp = rem[m:m + NCORES]
        c = grp[0][0]
        per = [(g[1], g[2] * 128) for g in grp]
        per += [None] * (NCORES - len(per))
        entries.append(({"kind": "mix", "c": c, "L": 8 * c}, per))

    # Half the vonly slots first (they need only WV + V data: the PE starts
    # ~3us in), half last (their 2-op chain overlaps the softmax drain of
    # the final regular slots); regular slots wide->narrow in between.
    von = [e for e in entries if e[0]["kind"] == "vonly"]
    reg = sorted((e for e in entries if e[0]["kind"] != "vonly"),
                 key=lambda e: -e[0]["L"])
    nf = (len(von) + 1) // 2
    entries = von[:nf] + reg + von[nf:]

    slots = [e[0] for e in entries]
    assign = [[e[1][i] for e in entries] for i in range(NCORES)]
    off = 0
    moff = 0
    for s in slots:
        s["off"] = off
        off += s["L"]
        if s["kind"] == "mix":
            s["moff"] = moff
            moff += s["L"]
    return slots, assign, off, moff


def _build(slots, total_L, mix_L):
    import concourse.bacc as bacc
    import concourse.mybir as mybir
    from concourse.tile import TileContext

    f32 = mybir.dt.float32
    f16 = mybir.dt.float16
    bf16 = mybir.dt.bfloat16
    AX = mybir.AxisListType
    OP = mybir.AluOpType
    ACTF = mybir.ActivationFunctionType

    nslot = len(slots)
    nvf = 0
    while nvf < nslot and slots[nvf]["kind"] == "vonly":
        nvf += 1
    nvb = 0
    while nvb < nslot - nvf and slots[nslot - 1 - nvb]["kind"] == "vonly":
        nvb += 1

    nc = bacc.Bacc()

    qg = nc.declare_dram_parameter("qg", [EMB, nslot * 128], f16, isOutput=False)
    kg = nc.declare_dram_parameter("kg", [EMB, nslot * 128], f16, isOutput=False)
    vg = nc.declare_dram_parameter("vg", [EMB, nslot * 128], f16, isOutput=False)
    wq = nc.declare_dram_parameter("wq", [EMB, EMB], f16, isOutput=False)
    wk = nc.declare_dram_parameter("wk", [EMB, EMB], f16, isOutput=False)
    wv = nc.declare_dram_parameter("wv", [EMB, EMB], f16, isOutput=False)
    qm = nc.declare_dram_parameter("qm", [128, nslot], f32, isOutput=False)
    pm = (nc.declare_dram_parameter("pm", [128, mix_L], bf16, isOutput=False)
          if mix_L else None)
    outp = nc.declare_dram_parameter("outp", [128, total_L], bf16, isOutput=True)

    nsup = -(-nslot // SUP)
    SUPW = SUP * 128

    with TileContext(nc) as tc:
        with (
            tc.tile_pool(name="consts", bufs=1) as cpool,
            tc.tile_pool(name="xin", bufs=2) as xpool,
            tc.tile_pool(name="psq2", bufs=2, space="PSUM") as qpool,
            tc.tile_pool(name="psk2", bufs=2, space="PSUM") as kpool,
            tc.tile_pool(name="psv4", bufs=4, space="PSUM") as vpool,
            tc.tile_pool(name="work", bufs=3) as wpool,
            tc.tile_pool(name="live", bufs=4) as lpool,
            tc.tile_pool(name="outs", bufs=2) as opool,
            tc.tile_pool(name="stats", bufs=4) as spool,
        ):
            qm_sb = cpool.tile([128, nslot], f32, tag="qm")
            nc.sync.dma_start(out=qm_sb[:], in_=qm[:, :])

            w_sb = {}

            def load_w(name, src):
                t = cpool.tile([128, KC * EMB], f16, tag=name, name=name)
                nc.sync.dma_start(
                    out=t[:].rearrange("p (k c) -> p k c", k=KC),
                    in_=src[:, :].rearrange("(k p) c -> p k c", p=128),
                )
                w_sb[name] = t

            def w_ap(name, kc, c):
                blk = w_sb[name][:, kc * EMB:(kc + 1) * EMB]
                if c == 64:
                    return blk
                return blk.rearrange("p (h j) -> p h j", j=D)[:, :, :c]

            def load_sup_piece(tiles, sup, name, src, s0, s1):
                # load slots [s0, s1) of this sup for one tensor: 1 trigger
                lo = sup * SUP
                cols = (s1 - s0) * 128
                tcol = (s0 - lo) * 128
                dst = tiles[name][:].rearrange(
                    "p (k c) -> p k c", k=KC)[:, :, tcol:tcol + cols]
                nc.sync.dma_start(
                    out=dst,
                    in_=src[:, s0 * 128:s1 * 128]
                    .rearrange("(k p) c -> p k c", p=128),
                )

            def sup_tiles():
                return {name: xpool.tile([128, KC * SUPW], f16,
                                         tag=f"x{name}", name=f"x{name}")
                        for name in ("v", "k", "q")}

            def load_sup(sup):
                lo, hi = sup * SUP, min((sup + 1) * SUP, nslot)
                tiles = sup_tiles()
                load_sup_piece(tiles, sup, "v", vg, lo, hi)
                qk_lo, qk_hi = max(lo, nvf), min(hi, nslot - nvb)
                if qk_lo < qk_hi:
                    load_sup_piece(tiles, sup, "k", kg, qk_lo, qk_hi)
                    load_sup_piece(tiles, sup, "q", qg, qk_lo, qk_hi)
                return tiles

            def x_ap(tiles, name, kc, j):
                col = kc * SUPW + (j % SUP) * 128
                return tiles[name][:, col:col + 128]

            # Startup-critical order: WV + leading-vonly V data first (PE
            # starts ~3us in), then the first two regular slots' data
            # interleaved with the Q/K weights, then the rest of sup 0.
            load_w("wv", wv)
            xs0 = sup_tiles()
            hi0 = min(SUP, nslot)
            if nvf:
                load_sup_piece(xs0, 0, "v", vg, 0, min(nvf, hi0))
            r0 = min(nvf, hi0)
            r1 = min(r0 + 2, hi0)
            if r1 > r0:
                load_sup_piece(xs0, 0, "v", vg, r0, r1)
            load_w("wk", wk)
            if r1 > r0:
                load_sup_piece(xs0, 0, "k", kg, r0, r1)
            load_w("wq", wq)
            if r1 > r0:
                load_sup_piece(xs0, 0, "q", qg, r0, r1)
            if hi0 > r1:
                load_sup_piece(xs0, 0, "v", vg, r1, hi0)
                load_sup_piece(xs0, 0, "k", kg, r1, hi0)
                load_sup_piece(xs0, 0, "q", qg, r1, hi0)
            if pm is not None:
                pm_sb = cpool.tile([128, mix_L], bf16, tag="pm")
                nc.sync.dma_start(out=pm_sb[:], in_=pm[:, :])

            # Output staging: one [128, supL] tile per sup, single store
            # trigger per sup from the ACT queue.
            sup_lo = [sp * SUP for sp in range(nsup)]
            sup_hi = [min((sp + 1) * SUP, nslot) for sp in range(nsup)]
            supL = [sum(slots[j]["L"] for j in range(sup_lo[sp], sup_hi[sp]))
                    for sp in range(nsup)]
            oloc = {}
            for sp in range(nsup):
                col = 0
                for j in range(sup_lo[sp], sup_hi[sp]):
                    oloc[j] = col
                    col += slots[j]["L"]
            max_supL = max(supL)
            otiles = {}

            def o_slice(j):
                sp = j // SUP
                if sp not in otiles:
                    otiles[sp] = opool.tile([128, max_supL], bf16, tag="osup",
                                            name="osup")
                return otiles[sp][:, oloc[j]:oloc[j] + slots[j]["L"]]

            def store_sup(sp):
                g0 = slots[sup_lo[sp]]["off"]
                nc.scalar.dma_start(out=outp[:, g0:g0 + supL[sp]],
                                    in_=otiles[sp][:, :supL[sp]])

            def front(j, xs):
                s = slots[j]
                c, L, kind = s["c"], s["L"], s["kind"]
                psv = vpool.tile([128, EMB], f32, tag="psv")
                for kc in range(KC):
                    nc.tensor.matmul(
                        psv[:, :L], x_ap(xs, "v", kc, j), w_ap("wv", kc, c),
                        start=(kc == 0), stop=(kc == KC - 1),
                    )
                if kind == "vonly":
                    nc.scalar.activation(
                        o_slice(j), psv[:, :L], ACTF.Copy,
                        scale=qm_sb[:, j:j + 1],
                    )
                    return None
                psk = kpool.tile([128, EMB], f32, tag="psk")
                psq = qpool.tile([128, EMB], f32, tag="psq")
                for ps, xn, wn in ((psk, "k", "wk"), (psq, "q", "wq")):
                    for kc in range(KC):
                        nc.tensor.matmul(
                            ps[:, :L], x_ap(xs, xn, kc, j), w_ap(wn, kc, c),
                            start=(kc == 0), stop=(kc == KC - 1),
                        )
                # DVE may read at most one PSUM operand: stage K via SBUF
                k_sb = wpool.tile([128, EMB], f32, tag="k_sb")
                nc.scalar.copy(k_sb[:, :L], psk[:, :L])
                a = wpool.tile([128, EMB], f32, tag="a")
                nc.vector.tensor_mul(a[:, :L], psq[:, :L], k_sb[:, :L])
                if kind == "mix":
                    moff = s["moff"]
                    am = wpool.tile([128, EMB], f32, tag="am")
                    nc.vector.scalar_tensor_tensor(
                        am[:, :L], pm_sb[:, moff:moff + L], -10000.0,
                        a[:, :L], op0=OP.mult, op1=OP.add,
                    )
                    a = am
                mneg = spool.tile([128, H], f32, tag="mneg")
                av = a[:, :L].rearrange("p (g j) -> p g j", j=c)
                nc.vector.tensor_reduce(mneg[:], av, axis=AX.X, op=OP.max,
                                        negate=True)
                t_m = wpool.tile([128, EMB], f32, tag="t_m")
                mneg_b = (mneg[:].rearrange("p (g o) -> p g o", o=1)
                          .broadcast_to((128, H, c)))
                nc.gpsimd.tensor_add(
                    t_m[:, :L].rearrange("p (g j) -> p g j", j=c), av, mneg_b)
                e = lpool.tile([128, EMB], bf16, tag="e")
                # Q_len row mask rides the exp bias: dead rows get -1e4 so
                # e == 0 there (the resulting 0*inf NaNs in dead rows are
                # zeroed by the host scatter).
                nc.scalar.activation(e[:, :L], t_m[:, :L], ACTF.Exp,
                                     bias=qm_sb[:, j:j + 1])
                return e, psv

            def back(j, e, psv):
                s = slots[j]
                c, L = s["c"], s["L"]
                ev = e[:, :L].rearrange("p (g j) -> p g j", j=c)
                ssum = spool.tile([128, H], f32, tag="ssum")
                nc.vector.tensor_reduce(ssum[:], ev, axis=AX.X, op=OP.add)
                r = spool.tile([128, H], bf16, tag="r")
                with nc.allow_low_precision(reason="1/S at bf16: ~0.4% on softmax weights, well under the 2e-2 gate"):
                    nc.vector.reciprocal(r[:], ssum[:])
                p = wpool.tile([128, EMB], bf16, tag="p")
                r_b = (r[:].rearrange("p (g o) -> p g o", o=1)
                       .broadcast_to((128, H, c)))
                nc.gpsimd.tensor_mul(
                    p[:, :L].rearrange("p (g j) -> p g j", j=c), ev, r_b)
                nc.vector.tensor_mul(o_slice(j), p[:, :L], psv[:, :L])

            done = set()

            def flush(j):
                done.add(j)
                sp = j // SUP
                if sp in otiles and all(k in done
                                        for k in range(sup_lo[sp], sup_hi[sp])):
                    store_sup(sp)
                    del otiles[sp]

            xs_cur = xs0
            pending = None
            for j in range(nslot + 1):
                if j < nslot:
                    sup, local = divmod(j, SUP)
                    if local == 0 and sup > 0:
                        xs_cur = load_sup(sup)
                    res = front(j, xs_cur)
                else:
                    res = None
                if pending is not None:
                    back(pending[0], *pending[1])
                    flush(pending[0])
                    pending = None
                if j < nslot:
                    if res is None:
                        flush(j)
                    else:
                        pending = (j, res)

    nc.finalize()
    return nc


def _prep_inputs(Q_seq, K_seq, V_seq, Q_len, V_len, WQ, WK, WV):
    slots, assign, total_L, mix_L = _plan(Q_len, V_len)
    f16 = np.float16
    bf = ml_dtypes.bfloat16
    nslot = len(slots)

    wq_h = np.ascontiguousarray((WQ * 0.125).astype(f16))
    wk_h = np.ascontiguousarray(WK.astype(f16))
    wv_h = np.ascontiguousarray(WV.astype(f16))

    need_qk = {ba for i in range(NCORES) for j, s in enumerate(slots)
               if s["kind"] != "vonly" and assign[i][j] is not None
               for ba in [assign[i][j][0]]}
    need_v = {ba for i in range(NCORES) for j in range(nslot)
              if assign[i][j] is not None
              for ba in [assign[i][j][0]]}
    qT = {b: np.ascontiguousarray(Q_seq[b].T.astype(f16)) for b in need_qk}
    kT = {b: np.ascontiguousarray(K_seq[b].T.astype(f16)) for b in need_qk}
    vT = {b: np.ascontiguousarray(V_seq[b].T.astype(f16)) for b in need_v}

    in_maps = []
    for i in range(NCORES):
        qg = np.zeros((EMB, nslot * 128), f16)
        kg = np.zeros((EMB, nslot * 128), f16)
        vg = np.zeros((EMB, nslot * 128), f16)
        qmv = np.zeros((128, nslot), np.float32)
        pmv = np.zeros((128, mix_L), bf) if mix_L else None
        for j, s in enumerate(slots):
            ent = assign[i][j]
            if ent is None:
                continue
            b, tok0 = ent
            cs = slice(j * 128, (j + 1) * 128)
            ts = slice(tok0, tok0 + 128)
            vg[:, cs] = vT[b][:, ts]
            ql = int(Q_len[b, 0])
            live = int(np.clip(ql - tok0, 0, 128))
            if s["kind"] == "vonly":
                # multiplicative scale on the V copy (folds the 1/64)
                qmv[:live, j] = 1.0 / 64
            else:
                # additive exp bias: -1e4 on dead rows zeroes e there
                qmv[live:, j] = -1e4
            if s["kind"] != "vonly":
                qg[:, cs] = qT[b][:, ts]
                kg[:, cs] = kT[b][:, ts]
            if s["kind"] == "mix":
                vl = int(V_len[b, 0])
                c = s["c"]
                if vl < c:
                    dead = np.zeros((H, c), np.float32)
                    dead[:, vl:] = 1.0
                    pmv[:, s["moff"]:s["moff"] + s["L"]] = \
                        np.broadcast_to(dead.reshape(-1), (128, s["L"]))
        m = {
            "qg": qg, "kg": kg, "vg": vg,
            "wq": wq_h, "wk": wk_h, "wv": wv_h,
            "qm": np.ascontiguousarray(qmv),
        }
        if mix_L:
            m["pm"] = np.ascontiguousarray(pmv)
        in_maps.append(m)
    return in_maps, slots, assign, total_L


def _run(inputs, trace=False, mm_dtype_name="", tmpdir=None):
    from concourse.bass_utils import run_bass_kernel_spmd

    Q_len = np.asarray(inputs["Q_len"])
    V_len = np.asarray(inputs["V_len"])
    in_maps, slots, assign, total_L = _prep_inputs(
        np.asarray(inputs["Q_seq"]), np.asarray(inputs["K_seq"]),
        np.asarray(inputs["V_seq"]), Q_len, V_len,
        np.asarray(inputs["WQ"]), np.asarray(inputs["WK"]),
        np.asarray(inputs["WV"]))

    key = tuple((s["kind"], s["L"]) for s in slots)
    if key not in _CACHE:
        mix_L = sum(s["L"] for s in slots if s["kind"] == "mix")
        _CACHE[key] = _build(slots, total_L, mix_L)
    nc = _CACHE[key]

    res = run_bass_kernel_spmd(nc, in_maps, core_ids=list(range(NCORES)),
                               trace=trace, tmpdir=tmpdir)

    out = np.zeros((B, S, H * D), np.float32)
    for i in range(NCORES):
        po = res.results[i]["outp"].astype(np.float32)
        for j, s in enumerate(slots):
            ent = assign[i][j]
            if ent is None:
                continue
            b, tok0 = ent
            c, L, off = s["c"], s["L"], s["off"]
            live = int(np.clip(int(Q_len[b, 0]) - tok0, 0, 128))
            block = po[:live, off:off + L].reshape(live, H, c)
            if s["kind"] == "vonly":
                out[b, tok0:tok0 + live] = block.reshape(live, H * D)
            else:
                vl = int(V_len[b, 0])
                out[b, tok0:tok0 + live].reshape(live, H, D)[:, :, :vl] = \
                    block[:, :, :vl]
    return out, res


def kernel(Q_seq, K_seq, V_seq, Q_len, V_len, WQ, WK, WV):
    out, _ = _run(dict(Q_seq=Q_seq, K_seq=K_seq, V_seq=V_seq,
                       Q_len=Q_len, V_len=V_len, WQ=WQ, WK=WK, WV=WV))
    return out


# revision 21
# speedup vs baseline: 1.0040x; 1.0040x over previous
"""Trainium2 Bass kernel for nn_Attention_558345749040.

Reference (per batch b, H=8 heads of d=64, S=4096, E=512):
    Q = Q_seq @ WQ ; K = K_seq @ WK ; V = V_seq @ WV
    A = (Q * K) / 8                      (elementwise)
    softmax over each head's 64-wide feature group, positions j >= V_len[b]
    masked out (V_len == 0 degenerates to a uniform 1/64 softmax)
    O = softmax * V, rows s >= Q_len[b] zeroed

Structure exploited (all derived from the runtime Q_len / V_len values, so
the compiled schedule is input-shape-specialized but value-generic):
  * Rows s >= Q_len[b] are zero: only ceil(Q_len/128) 128-token chunks per
    batch carry live data. Live chunks are repartitioned evenly across the
    8 cores (token-balanced data parallel), removing the Q_len imbalance.
  * Only head positions j < V_len[b] matter: the Q/K/V matmuls select the
    8*V_len live weight columns through a strided moving AP over the shared
    full weight tiles (PE matmul cost scales with output free size), the
    softmax runs on vl-wide groups, and only packed columns are stored; the
    host scatters them back into a zero canvas. Full-quota slots need no
    masking at all; remainder chunks share mixed-width slots and get an
    additive -1e4 pre-softmax mask (fused multiply-add, one DVE op).
  * V_len == 0 batches reduce to O = V/64: V-matmul-only slots.
  * fp16 transport + fp16 matmuls throughout (measured rel err 3.6e-3 vs
    the 2e-2 gate; bf16 Q/K fails at 2.5e-2, fp8 V fails at 3.7e-2).
  * HWDGE DMA triggers cost a flat ~625ns on one shared generator: loads
    are batched 4-contraction-chunks-per-trigger, stores one per superslot
    (issued from the ACT queue so input loads never queue behind them).
  * The Q_len row mask rides the softmax-weight multiply as a per-partition
    scalar (fused (e*qm)*r), so V flows from PSUM straight into the final
    elementwise multiply with no staging copy.

Every core runs the same instruction stream (SPMD single-NEFF constraint):
the slot schedule (widths/kinds) is identical across cores; which batch
chunk a slot processes is pure data (gathered inputs + per-slot masks).
"""

import numpy as np
import ml_dtypes

B, S, EMB = 8, 4096, 512
H, D = 8, 64
NCORES = 8
KC = EMB // 128          # 4 contraction chunks
SUP = 8                  # slots per input-DMA superslot

_CACHE = {}


def _plan(Q_len, V_len):
    """Slot schedule shared by all cores + per-core chunk assignment.

    Returns (slots, assign, total_L, mix_L) where slots[j] holds
    {kind: 'reg'|'mix'|'vonly', c, L, off, moff} and assign[i][j] is
    (batch, tok0) for the chunk core i processes in slot j (None = dummy).
    """
    entries = []  # (slotdict, percore list)

    def chunks_of(b):
        ql = int(Q_len[b, 0])
        return -(-ql // 128) if ql > 0 else 0

    rem = []
    for b in range(B):
        nch = chunks_of(b)
        if nch == 0:
            continue
        vl = int(V_len[b, 0])
        if vl == 0:
            quota = -(-nch // NCORES)
            for t in range(quota):
                per = [(b, (t * NCORES + i) * 128)
                       if t * NCORES + i < nch else None
                       for i in range(NCORES)]
                entries.append(({"kind": "vonly", "c": D, "L": 8 * D}, per))
        else:
            fq = nch // NCORES
            for t in range(fq):
                per = [(b, (t * NCORES + i) * 128) for i in range(NCORES)]
                entries.append(({"kind": "reg", "c": vl, "L": 8 * vl}, per))
            for ch in range(NCORES * fq, nch):
                rem.append((vl, b, ch))

    # Remainder chunks: sort by width desc, deal round-robin into mixed
    # slots whose width is the max of their 8 chunks (others get a -1e4
    # additive mask on the padding columns).
    rem.sort(key=lambda x: -x[0])
    for m in range(0, len(rem), NCORES):
        grp = rem[m:m + NCORES]
        c = grp[0][0]
        per = [(g[1], g[2] * 128) for g in grp]
        per += [None] * (NCORES - len(per))
        entries.append(({"kind": "mix", "c": c, "L": 8 * c}, per))

    # vonly slots first (they need only WV + V data: the PE starts ~3us
    # in), then regular slots wide->narrow so the tail drains on cheap
    # slots.
    von = [e for e in entries if e[0]["kind"] == "vonly"]
    reg = sorted((e for e in entries if e[0]["kind"] != "vonly"),
                 key=lambda e: -e[0]["L"])
    entries = von + reg

    slots = [e[0] for e in entries]
    assign = [[e[1][i] for e in entries] for i in range(NCORES)]
    off = 0
    moff = 0
    for s in slots:
        s["off"] = off
        off += s["L"]
        if s["kind"] == "mix":
            s["moff"] = moff
            moff += s["L"]
    return slots, assign, off, moff


def _build(slots, total_L, mix_L):
    import concourse.bacc as bacc
    import concourse.mybir as mybir
    from concourse.tile import TileContext

    f32 = mybir.dt.float32
    f16 = mybir.dt.float16
    bf16 = mybir.dt.bfloat16
    AX = mybir.AxisListType
    OP = mybir.AluOpType
    ACTF = mybir.ActivationFunctionType

    nslot = len(slots)
    nvf = 0
    while nvf < nslot and slots[nvf]["kind"] == "vonly":
        nvf += 1
    nvb = 0
    while nvb < nslot - nvf and slots[nslot - 1 - nvb]["kind"] == "vonly":
        nvb += 1

    nc = bacc.Bacc()

    qg = nc.declare_dram_parameter("qg", [EMB, nslot * 128], f16, isOutput=False)
    kg = nc.declare_dram_parameter("kg", [EMB, nslot * 128], f16, isOutput=False)
    vg = nc.declare_dram_parameter("vg", [EMB, nslot * 128], f16, isOutput=False)
    wq = nc.declare_dram_parameter("wq", [EMB, EMB], f16, isOutput=False)
    wk = nc.declare_dram_parameter("wk", [EMB, EMB], f16, isOutput=False)
    wv = nc.declare_dram_parameter("wv", [EMB, EMB], f16, isOutput=False)
    qm = nc.declare_dram_parameter("qm", [128, nslot], f32, isOutput=False)
    pm = (nc.declare_dram_parameter("pm", [128, mix_L], bf16, isOutput=False)
          if mix_L else None)
    outp = nc.declare_dram_parameter("outp", [128, total_L], bf16, isOutput=True)

    nsup = -(-nslot // SUP)
    SUPW = SUP * 128

    with TileContext(nc) as tc:
        with (
            tc.tile_pool(name="consts", bufs=1) as cpool,
            tc.tile_pool(name="xin", bufs=2) as xpool,
            tc.tile_pool(name="psq2", bufs=2, space="PSUM") as qpool,
            tc.tile_pool(name="psk2", bufs=2, space="PSUM") as kpool,
            tc.tile_pool(name="psv2x", bufs=2, space="PSUM") as vpool,
            tc.tile_pool(name="work", bufs=3) as wpool,
            tc.tile_pool(name="live", bufs=4) as lpool,
            tc.tile_pool(name="outs", bufs=2) as opool,
            tc.tile_pool(name="stats", bufs=4) as spool,
        ):
            qm_sb = cpool.tile([128, nslot], f32, tag="qm")
            nc.sync.dma_start(out=qm_sb[:], in_=qm[:, :])

            w_sb = {}

            def load_w(name, src):
                t = cpool.tile([128, KC * EMB], f16, tag=name, name=name)
                nc.sync.dma_start(
                    out=t[:].rearrange("p (k c) -> p k c", k=KC),
                    in_=src[:, :].rearrange("(k p) c -> p k c", p=128),
                )
                w_sb[name] = t

            def w_ap(name, kc, c):
                blk = w_sb[name][:, kc * EMB:(kc + 1) * EMB]
                if c == 64:
                    return blk
                return blk.rearrange("p (h j) -> p h j", j=D)[:, :, :c]

            def load_sup_piece(tiles, sup, name, src, s0, s1):
                # load slots [s0, s1) of this sup for one tensor: 1 trigger
                lo = sup * SUP
                cols = (s1 - s0) * 128
                tcol = (s0 - lo) * 128
                dst = tiles[name][:].rearrange(
                    "p (k c) -> p k c", k=KC)[:, :, tcol:tcol + cols]
                nc.sync.dma_start(
                    out=dst,
                    in_=src[:, s0 * 128:s1 * 128]
                    .rearrange("(k p) c -> p k c", p=128),
                )

            def sup_tiles():
                return {name: xpool.tile([128, KC * SUPW], f16,
                                         tag=f"x{name}", name=f"x{name}")
                        for name in ("v", "k", "q")}

            def load_sup(sup):
                lo, hi = sup * SUP, min((sup + 1) * SUP, nslot)
                tiles = sup_tiles()
                load_sup_piece(tiles, sup, "v", vg, lo, hi)
                qk_lo, qk_hi = max(lo, nvf), min(hi, nslot - nvb)
                if qk_lo < qk_hi:
                    load_sup_piece(tiles, sup, "k", kg, qk_lo, qk_hi)
                    load_sup_piece(tiles, sup, "q", qg, qk_lo, qk_hi)
                return tiles

            def x_ap(tiles, name, kc, j):
                col = kc * SUPW + (j % SUP) * 128
                return tiles[name][:, col:col + 128]

            # Startup-critical order: WV + leading-vonly V data first (PE
            # starts ~3us in), then the first two regular slots' data
            # interleaved with the Q/K weights, then the rest of sup 0.
            load_w("wv", wv)
            xs0 = sup_tiles()
            hi0 = min(SUP, nslot)
            if nvf:
                load_sup_piece(xs0, 0, "v", vg, 0, min(nvf, hi0))
            r0 = min(nvf, hi0)
            r1 = min(r0 + 2, hi0)
            if r1 > r0:
                load_sup_piece(xs0, 0, "v", vg, r0, r1)
            load_w("wk", wk)
            if r1 > r0:
                load_sup_piece(xs0, 0, "k", kg, r0, r1)
            load_w("wq", wq)
            if r1 > r0:
                load_sup_piece(xs0, 0, "q", qg, r0, r1)
            if hi0 > r1:
                load_sup_piece(xs0, 0, "v", vg, r1, hi0)
                load_sup_piece(xs0, 0, "k", kg, r1, hi0)
                load_sup_piece(xs0, 0, "q", qg, r1, hi0)
            if pm is not None:
                pm_sb = cpool.tile([128, mix_L], bf16, tag="pm")
                nc.sync.dma_start(out=pm_sb[:], in_=pm[:, :])

            # Output staging: one [128, supL] tile per sup, single store
            # trigger per sup from the ACT queue.
            sup_lo = [sp * SUP for sp in range(nsup)]
            sup_hi = [min((sp + 1) * SUP, nslot) for sp in range(nsup)]
            supL = [sum(slots[j]["L"] for j in range(sup_lo[sp], sup_hi[sp]))
                    for sp in range(nsup)]
            oloc = {}
            for sp in range(nsup):
                col = 0
                for j in range(sup_lo[sp], sup_hi[sp]):
                    oloc[j] = col
                    col += slots[j]["L"]
            max_supL = max(supL)
            otiles = {}

            def o_slice(j):
                sp = j // SUP
                if sp not in otiles:
                    otiles[sp] = opool.tile([128, max_supL], bf16, tag="osup",
                                            name="osup")
                return otiles[sp][:, oloc[j]:oloc[j] + slots[j]["L"]]

            def store_sup(sp):
                g0 = slots[sup_lo[sp]]["off"]
                nc.scalar.dma_start(out=outp[:, g0:g0 + supL[sp]],
                                    in_=otiles[sp][:, :supL[sp]])

            # Same-width neighbor slots (within one sup) share a 2-bank
            # PSUM V tile and run the softmax chain as pair-wide ops —
            # halves the instruction count on DVE/Pool for those stages.
            items = []
            jj = 0
            while jj < nslot:
                if (jj + 1 < nslot and jj // SUP == (jj + 1) // SUP
                        and slots[jj]["c"] == slots[jj + 1]["c"]
                        and ((slots[jj]["kind"] == "vonly")
                             == (slots[jj + 1]["kind"] == "vonly"))):
                    items.append((jj, jj + 1))
                    jj += 2
                else:
                    items.append((jj,))
                    jj += 1

            def wide(x, nu, c):
                # [p, (u), g, j] view of a 512-col-strided pair tile
                if nu == 2:
                    return (x[:].rearrange("p (u y) -> p u y", u=2)
                            [:, :, :8 * c]
                            .rearrange("p u (g j) -> p u g j", j=c))
                return x[:, :8 * c].rearrange("p (g j) -> p g j", j=c)

            def stat(x, nu):
                if nu == 2:
                    return x[:].rearrange("p (u g) -> p u g", u=2)
                return x[:, :H]

            def stat_b(x, nu, c):
                if nu == 2:
                    return (x[:].rearrange("p (u g o) -> p u g o", u=2, o=1)
                            .broadcast_to((128, 2, H, c)))
                return (x[:, :H].rearrange("p (g o) -> p g o", o=1)
                        .broadcast_to((128, H, c)))

            def front(js, xs):
                nu = len(js)
                c = slots[js[0]]["c"]
                L = 8 * c
                psv = vpool.tile([128, 2 * EMB], f32, tag="psv")
                for h, j in enumerate(js):
                    hs = psv[:, h * EMB:h * EMB + L]
                    for kc in range(KC):
                        nc.tensor.matmul(
                            hs, x_ap(xs, "v", kc, j), w_ap("wv", kc, c),
                            start=(kc == 0), stop=(kc == KC - 1),
                        )
                if slots[js[0]]["kind"] == "vonly":
                    for h, j in enumerate(js):
                        nc.scalar.activation(
                            o_slice(j), psv[:, h * EMB:h * EMB + L],
                            ACTF.Copy, scale=qm_sb[:, j:j + 1],
                        )
                    return None
                a = wpool.tile([128, 2 * EMB], f32, tag="a")
                for h, j in enumerate(js):
                    psk = kpool.tile([128, EMB], f32, tag="psk")
                    psq = qpool.tile([128, EMB], f32, tag="psq")
                    for ps, xn, wn in ((psk, "k", "wk"), (psq, "q", "wq")):
                        for kc in range(KC):
                            nc.tensor.matmul(
                                ps[:, :L], x_ap(xs, xn, kc, j),
                                w_ap(wn, kc, c),
                                start=(kc == 0), stop=(kc == KC - 1),
                            )
                    # DVE reads at most one PSUM operand: stage K via SBUF
                    k_sb = wpool.tile([128, EMB], f32, tag="k_sb")
                    nc.scalar.copy(k_sb[:, :L], psk[:, :L])
                    ah = a[:, h * EMB:h * EMB + L]
                    nc.vector.tensor_mul(ah, psq[:, :L], k_sb[:, :L])
                    if slots[j]["kind"] == "mix":
                        moff = slots[j]["moff"]
                        nc.vector.scalar_tensor_tensor(
                            ah, pm_sb[:, moff:moff + L], -10000.0,
                            ah, op0=OP.mult, op1=OP.add,
                        )
                av = wide(a, nu, c)
                mneg = spool.tile([128, 2 * H], f32, tag="mneg")
                nc.vector.tensor_reduce(stat(mneg, nu), av, axis=AX.X,
                                        op=OP.max, negate=True)
                t_m = wpool.tile([128, 2 * EMB], f32, tag="t_m")
                nc.gpsimd.tensor_add(wide(t_m, nu, c), av, stat_b(mneg, nu, c))
                e = lpool.tile([128, 2 * EMB], bf16, tag="e")
                # Q_len row mask rides the exp bias: dead rows get -1e4 so
                # e == 0 there (the resulting 0*inf NaNs in dead rows are
                # zeroed by the host scatter).
                for h, j in enumerate(js):
                    nc.scalar.activation(
                        e[:, h * EMB:h * EMB + L], t_m[:, h * EMB:h * EMB + L],
                        ACTF.Exp, bias=qm_sb[:, j:j + 1])
                return e, psv

            def back(js, e, psv):
                nu = len(js)
                c = slots[js[0]]["c"]
                L = 8 * c
                ev = wide(e, nu, c)
                ssum = spool.tile([128, 2 * H], f32, tag="ssum")
                nc.vector.tensor_reduce(stat(ssum, nu), ev, axis=AX.X,
                                        op=OP.add)
                r = spool.tile([128, 2 * H], bf16, tag="r")
                with nc.allow_low_precision(reason="1/S at bf16: ~0.4% on softmax weights, well under the 2e-2 gate"):
                    nc.vector.reciprocal(stat(r, nu), stat(ssum, nu))
                p = wpool.tile([128, 2 * EMB], bf16, tag="p")
                nc.gpsimd.tensor_mul(wide(p, nu, c), ev, stat_b(r, nu, c))
                loc0 = oloc[js[0]]
                sp = js[0] // SUP
                ov = otiles[sp][:, loc0:loc0 + nu * L]
                if nu == 2:
                    ov = ov.rearrange("p (u g j) -> p u g j", u=2, j=c)
                else:
                    ov = ov.rearrange("p (g j) -> p g j", j=c)
                nc.vector.tensor_mul(ov, wide(p, nu, c), wide(psv, nu, c))

            done = set()

            def flush(j):
                done.add(j)
                sp = j // SUP
                if sp in otiles and all(k in done
                                        for k in range(sup_lo[sp], sup_hi[sp])):
                    store_sup(sp)
                    del otiles[sp]

            xs_cur = xs0
            pending = None
            for it in list(items) + [None]:
                if it is not None:
                    sup, local = divmod(it[0], SUP)
                    if local == 0 and sup > 0:
                        xs_cur = load_sup(sup)
                    for j in it:
                        o_slice(j)  # ensure the sup's otile exists
                    res = front(it, xs_cur)
                else:
                    res = None
                if pending is not None:
                    back(pending[0], *pending[1])
                    for j in pending[0]:
                        flush(j)
                    pending = None
                if it is not None:
                    if res is None:
                        for j in it:
                            flush(j)
                    else:
                        pending = (it, res)

    nc.finalize()
    return nc


def _prep_inputs(Q_seq, K_seq, V_seq, Q_len, V_len, WQ, WK, WV):
    slots, assign, total_L, mix_L = _plan(Q_len, V_len)
    f16 = np.float16
    bf = ml_dtypes.bfloat16
    nslot = len(slots)

    wq_h = np.ascontiguousarray((WQ * 0.125).astype(f16))
    wk_h = np.ascontiguousarray(WK.astype(f16))
    wv_h = np.ascontiguousarray(WV.astype(f16))

    need_qk = {ba for i in range(NCORES) for j, s in enumerate(slots)
               if s["kind"] != "vonly" and assign[i][j] is not None
               for ba in [assign[i][j][0]]}
    need_v = {ba for i in range(NCORES) for j in range(nslot)
              if assign[i][j] is not None
              for ba in [assign[i][j][0]]}
    qT = {b: np.ascontiguousarray(Q_seq[b].T.astype(f16)) for b in need_qk}
    kT = {b: np.ascontiguousarray(K_seq[b].T.astype(f16)) for b in need_qk}
    vT = {b: np.ascontiguousarray(V_seq[b].T.astype(f16)) for b in need_v}

    in_maps = []
    for i in range(NCORES):
        qg = np.zeros((EMB, nslot * 128), f16)
        kg = np.zeros((EMB, nslot * 128), f16)
        vg = np.zeros((EMB, nslot * 128), f16)
        qmv = np.zeros((128, nslot), np.float32)
        pmv = np.zeros((128, mix_L), bf) if mix_L else None
        for j, s in enumerate(slots):
            ent = assign[i][j]
            if ent is None:
                continue
            b, tok0 = ent
            cs = slice(j * 128, (j + 1) * 128)
            ts = slice(tok0, tok0 + 128)
            vg[:, cs] = vT[b][:, ts]
            ql = int(Q_len[b, 0])
            live = int(np.clip(ql - tok0, 0, 128))
            if s["kind"] == "vonly":
                # multiplicative scale on the V copy (folds the 1/64)
                qmv[:live, j] = 1.0 / 64
            else:
                # additive exp bias: -1e4 on dead rows zeroes e there
                qmv[live:, j] = -1e4
            if s["kind"] != "vonly":
                qg[:, cs] = qT[b][:, ts]
                kg[:, cs] = kT[b][:, ts]
            if s["kind"] == "mix":
                vl = int(V_len[b, 0])
                c = s["c"]
                if vl < c:
                    dead = np.zeros((H, c), np.float32)
                    dead[:, vl:] = 1.0
                    pmv[:, s["moff"]:s["moff"] + s["L"]] = \
                        np.broadcast_to(dead.reshape(-1), (128, s["L"]))
        m = {
            "qg": qg, "kg": kg, "vg": vg,
            "wq": wq_h, "wk": wk_h, "wv": wv_h,
            "qm": np.ascontiguousarray(qmv),
        }
        if mix_L:
            m["pm"] = np.ascontiguousarray(pmv)
        in_maps.append(m)
    return in_maps, slots, assign, total_L


def _run(inputs, trace=False, mm_dtype_name="", tmpdir=None):
    from concourse.bass_utils import run_bass_kernel_spmd

    Q_len = np.asarray(inputs["Q_len"])
    V_len = np.asarray(inputs["V_len"])
    in_maps, slots, assign, total_L = _prep_inputs(
        np.asarray(inputs["Q_seq"]), np.asarray(inputs["K_seq"]),
        np.asarray(inputs["V_seq"]), Q_len, V_len,
        np.asarray(inputs["WQ"]), np.asarray(inputs["WK"]),
        np.asarray(inputs["WV"]))

    key = tuple((s["kind"], s["L"]) for s in slots)
    if key not in _CACHE:
        mix_L = sum(s["L"] for s in slots if s["kind"] == "mix")
        _CACHE[key] = _build(slots, total_L, mix_L)
    nc = _CACHE[key]

    res = run_bass_kernel_spmd(nc, in_maps, core_ids=list(range(NCORES)),
                               trace=trace, tmpdir=tmpdir)

    out = np.zeros((B, S, H * D), np.float32)
    for i in range(NCORES):
        po = res.results[i]["outp"].astype(np.float32)
        for j, s in enumerate(slots):
            ent = assign[i][j]
            if ent is None:
                continue
            b, tok0 = ent
            c, L, off = s["c"], s["L"], s["off"]
            live = int(np.clip(int(Q_len[b, 0]) - tok0, 0, 128))
            block = po[:live, off:off + L].reshape(live, H, c)
            if s["kind"] == "vonly":
                out[b, tok0:tok0 + live] = block.reshape(live, H * D)
            else:
                vl = int(V_len[b, 0])
                out[b, tok0:tok0 + live].reshape(live, H, D)[:, :, :vl] = \
                    block[:, :, :vl]
    return out, res


def kernel(Q_seq, K_seq, V_seq, Q_len, V_len, WQ, WK, WV):
    out, _ = _run(dict(Q_seq=Q_seq, K_seq=K_seq, V_seq=V_seq,
                       Q_len=Q_len, V_len=V_len, WQ=WQ, WK=WK, WV=WV))
    return out


# revision 26
# speedup vs baseline: 1.0129x; 1.0089x over previous
"""Trainium2 Bass kernel for nn_Attention_558345749040.

Reference (per batch b, H=8 heads of d=64, S=4096, E=512):
    Q = Q_seq @ WQ ; K = K_seq @ WK ; V = V_seq @ WV
    A = (Q * K) / 8                      (elementwise)
    softmax over each head's 64-wide feature group, positions j >= V_len[b]
    masked out (V_len == 0 degenerates to a uniform 1/64 softmax)
    O = softmax * V, rows s >= Q_len[b] zeroed

Structure exploited (all derived from the runtime Q_len / V_len values, so
the compiled schedule is input-shape-specialized but value-generic):
  * Rows s >= Q_len[b] are zero: only ceil(Q_len/128) 128-token chunks per
    batch carry live data. Live chunks are repartitioned evenly across the
    8 cores (token-balanced data parallel), removing the Q_len imbalance.
  * Only head positions j < V_len[b] matter: the Q/K/V matmuls select the
    8*V_len live weight columns through a strided moving AP over the shared
    full weight tiles (PE matmul cost scales with output free size), the
    softmax runs on vl-wide groups, and only packed columns are stored; the
    host scatters them back into a zero canvas. Full-quota slots need no
    masking at all; remainder chunks share mixed-width slots and get an
    additive -1e4 pre-softmax mask (fused multiply-add, one DVE op).
  * V_len == 0 batches reduce to O = V/64: V-matmul-only slots.
  * fp16 transport + fp16 matmuls throughout (measured rel err 3.6e-3 vs
    the 2e-2 gate; bf16 Q/K fails at 2.5e-2, fp8 V fails at 3.7e-2).
  * HWDGE DMA triggers cost a flat ~625ns on one shared generator: loads
    are batched 4-contraction-chunks-per-trigger, stores one per superslot
    (issued from the ACT queue so input loads never queue behind them).
  * The Q_len row mask rides the softmax-weight multiply as a per-partition
    scalar (fused (e*qm)*r), so V flows from PSUM straight into the final
    elementwise multiply with no staging copy.

Every core runs the same instruction stream (SPMD single-NEFF constraint):
the slot schedule (widths/kinds) is identical across cores; which batch
chunk a slot processes is pure data (gathered inputs + per-slot masks).
"""

import numpy as np
import ml_dtypes

B, S, EMB = 8, 4096, 512
H, D = 8, 64
NCORES = 8
KC = EMB // 128          # 4 contraction chunks
SUP = 8                  # slots per input-DMA superslot

_CACHE = {}


def _plan(Q_len, V_len):
    """Slot schedule shared by all cores + per-core chunk assignment.

    Returns (slots, assign, total_L, mix_L) where slots[j] holds
    {kind: 'reg'|'mix'|'vonly', c, L, off, moff} and assign[i][j] is
    (batch, tok0) for the chunk core i processes in slot j (None = dummy).
    """
    entries = []  # (slotdict, percore list)

    def chunks_of(b):
        ql = int(Q_len[b, 0])
        return -(-ql // 128) if ql > 0 else 0

    rem = []
    for b in range(B):
        nch = chunks_of(b)
        if nch == 0:
            continue
        vl = int(V_len[b, 0])
        if vl == 0:
            quota = -(-nch // NCORES)
            for t in range(quota):
                per = [(b, (t * NCORES + i) * 128)
                       if t * NCORES + i < nch else None
                       for i in range(NCORES)]
                entries.append(({"kind": "vonly", "c": D, "L": 8 * D}, per))
        else:
            fq = nch // NCORES
            for t in range(fq):
                per = [(b, (t * NCORES + i) * 128) for i in range(NCORES)]
                entries.append(({"kind": "reg", "c": vl, "L": 8 * vl}, per))
            for ch in range(NCORES * fq, nch):
                rem.append((vl, b, ch))

    # Remainder chunks: sort by width desc, deal round-robin into mixed
    # slots whose width is the max of their 8 chunks (others get a -1e4
    # additive mask on the padding columns).
    rem.sort(key=lambda x: -x[0])
    for m in range(0, len(rem), NCORES):
        grp = rem[m:m + NCORES]
        c = grp[0][0]
        per = [(g[1], g[2] * 128) for g in grp]
        per += [None] * (NCORES - len(per))
        entries.append(({"kind": "mix", "c": c, "L": 8 * c}, per))

    # vonly slots first (they need only WV + V data: the PE starts ~3us
    # in), then regular slots wide->narrow so the pipeline tail drains on
    # cheap slots.
    von = [e for e in entries if e[0]["kind"] == "vonly"]
    reg = sorted((e for e in entries if e[0]["kind"] != "vonly"),
                 key=lambda e: -e[0]["L"])
    entries = von + reg

    slots = [e[0] for e in entries]
    assign = [[e[1][i] for e in entries] for i in range(NCORES)]
    off = 0
    moff = 0
    for s in slots:
        s["off"] = off
        off += s["L"]
        if s["kind"] == "mix":
            s["moff"] = moff
            moff += s["L"]
    return slots, assign, off, moff


def _build(slots, total_L, mix_L):
    import concourse.bacc as bacc
    import concourse.mybir as mybir
    from concourse.tile import TileContext

    f32 = mybir.dt.float32
    f16 = mybir.dt.float16
    bf16 = mybir.dt.bfloat16
    AX = mybir.AxisListType
    OP = mybir.AluOpType
    ACTF = mybir.ActivationFunctionType

    nslot = len(slots)
    nvf = 0
    while nvf < nslot and slots[nvf]["kind"] == "vonly":
        nvf += 1
    nvb = 0
    while nvb < nslot - nvf and slots[nslot - 1 - nvb]["kind"] == "vonly":
        nvb += 1

    nc = bacc.Bacc()

    qg = nc.declare_dram_parameter("qg", [EMB, nslot * 128], f16, isOutput=False)
    kg = nc.declare_dram_parameter("kg", [EMB, nslot * 128], f16, isOutput=False)
    vg = nc.declare_dram_parameter("vg", [EMB, nslot * 128], f16, isOutput=False)
    wq = nc.declare_dram_parameter("wq", [EMB, EMB], f16, isOutput=False)
    wk = nc.declare_dram_parameter("wk", [EMB, EMB], f16, isOutput=False)
    wv = nc.declare_dram_parameter("wv", [EMB, EMB], f16, isOutput=False)
    qm = nc.declare_dram_parameter("qm", [128, nslot], f32, isOutput=False)
    pm = (nc.declare_dram_parameter("pm", [128, mix_L], bf16, isOutput=False)
          if mix_L else None)
    outp = nc.declare_dram_parameter("outp", [128, total_L], bf16, isOutput=True)

    nsup = -(-nslot // SUP)
    SUPW = SUP * 128

    with TileContext(nc) as tc:
        with (
            tc.tile_pool(name="consts", bufs=1) as cpool,
            tc.tile_pool(name="xin", bufs=2) as xpool,
            tc.tile_pool(name="psq2", bufs=2, space="PSUM") as qpool,
            tc.tile_pool(name="psk2", bufs=2, space="PSUM") as kpool,
            tc.tile_pool(name="psv4", bufs=4, space="PSUM") as vpool,
            tc.tile_pool(name="work", bufs=3) as wpool,
            tc.tile_pool(name="live", bufs=4) as lpool,
            tc.tile_pool(name="outs", bufs=2) as opool,
            tc.tile_pool(name="stats", bufs=4) as spool,
        ):
            qm_sb = cpool.tile([128, nslot], f32, tag="qm")
            nc.sync.dma_start(out=qm_sb[:], in_=qm[:, :])

            w_sb = {}

            def load_w(name, src, kcs=None):
                # kc-split triggers let the first matmuls start as soon as
                # the kc=0 weight block lands (startup is DMA-burst-bound)
                if name not in w_sb:
                    w_sb[name] = cpool.tile([128, KC * EMB], f16, tag=name,
                                            name=name)
                t = w_sb[name]
                for kc in (range(KC) if kcs is None else kcs):
                    nc.sync.dma_start(
                        out=t[:, kc * EMB:(kc + 1) * EMB],
                        in_=src[kc * 128:(kc + 1) * 128, :],
                    )

            def w_ap(name, kc, c):
                blk = w_sb[name][:, kc * EMB:(kc + 1) * EMB]
                if c == 64:
                    return blk
                return blk.rearrange("p (h j) -> p h j", j=D)[:, :, :c]

            def load_sup_piece(tiles, sup, name, src, s0, s1):
                # load slots [s0, s1) of this sup for one tensor: 1 trigger
                lo = sup * SUP
                cols = (s1 - s0) * 128
                tcol = (s0 - lo) * 128
                dst = tiles[name][:].rearrange(
                    "p (k c) -> p k c", k=KC)[:, :, tcol:tcol + cols]
                nc.sync.dma_start(
                    out=dst,
                    in_=src[:, s0 * 128:s1 * 128]
                    .rearrange("(k p) c -> p k c", p=128),
                )

            def sup_tiles():
                return {name: xpool.tile([128, KC * SUPW], f16,
                                         tag=f"x{name}", name=f"x{name}")
                        for name in ("v", "k", "q")}

            def load_sup(sup):
                lo, hi = sup * SUP, min((sup + 1) * SUP, nslot)
                tiles = sup_tiles()
                load_sup_piece(tiles, sup, "v", vg, lo, hi)
                qk_lo, qk_hi = max(lo, nvf), min(hi, nslot - nvb)
                if qk_lo < qk_hi:
                    load_sup_piece(tiles, sup, "k", kg, qk_lo, qk_hi)
                    load_sup_piece(tiles, sup, "q", qg, qk_lo, qk_hi)
                return tiles

            def x_ap(tiles, name, kc, j):
                col = kc * SUPW + (j % SUP) * 128
                return tiles[name][:, col:col + 128]

            # Startup-critical order: WV + leading-vonly V data first (PE
            # starts ~3us in), then the first two regular slots' data
            # interleaved with the Q/K weights, then the rest of sup 0.
            xs0 = sup_tiles()
            hi0 = min(SUP, nslot)
            load_w("wv", wv, kcs=[0])
            if nvf:
                load_sup_piece(xs0, 0, "v", vg, 0, min(nvf, hi0))
            load_w("wv", wv, kcs=[1, 2, 3])
            if hi0 > nvf:
                load_sup_piece(xs0, 0, "v", vg, nvf, hi0)
            load_w("wk", wk, kcs=[0])
            if hi0 > nvf:
                load_sup_piece(xs0, 0, "k", kg, nvf, hi0)
            load_w("wq", wq, kcs=[0])
            if hi0 > nvf:
                load_sup_piece(xs0, 0, "q", qg, nvf, hi0)
            load_w("wk", wk, kcs=[1, 2, 3])
            load_w("wq", wq, kcs=[1, 2, 3])
            if pm is not None:
                pm_sb = cpool.tile([128, mix_L], bf16, tag="pm")
                nc.sync.dma_start(out=pm_sb[:], in_=pm[:, :])

            # Output staging: one [128, supL] tile per sup, single store
            # trigger per sup from the ACT queue.
            sup_lo = [sp * SUP for sp in range(nsup)]
            sup_hi = [min((sp + 1) * SUP, nslot) for sp in range(nsup)]
            supL = [sum(slots[j]["L"] for j in range(sup_lo[sp], sup_hi[sp]))
                    for sp in range(nsup)]
            oloc = {}
            for sp in range(nsup):
                col = 0
                for j in range(sup_lo[sp], sup_hi[sp]):
                    oloc[j] = col
                    col += slots[j]["L"]
            max_supL = max(supL)
            otiles = {}

            def o_slice(j):
                sp = j // SUP
                if sp not in otiles:
                    otiles[sp] = opool.tile([128, max_supL], bf16, tag="osup",
                                            name="osup")
                return otiles[sp][:, oloc[j]:oloc[j] + slots[j]["L"]]

            def store_sup(sp):
                g0 = slots[sup_lo[sp]]["off"]
                nc.scalar.dma_start(out=outp[:, g0:g0 + supL[sp]],
                                    in_=otiles[sp][:, :supL[sp]])

            def front(j, xs):
                s = slots[j]
                c, L, kind = s["c"], s["L"], s["kind"]
                psv = vpool.tile([128, EMB], f32, tag="psv")
                for kc in range(KC):
                    nc.tensor.matmul(
                        psv[:, :L], x_ap(xs, "v", kc, j), w_ap("wv", kc, c),
                        start=(kc == 0), stop=(kc == KC - 1),
                    )
                if kind == "vonly":
                    nc.scalar.activation(
                        o_slice(j), psv[:, :L], ACTF.Copy,
                        scale=qm_sb[:, j:j + 1],
                    )
                    return None
                psk = kpool.tile([128, EMB], f32, tag="psk")
                psq = qpool.tile([128, EMB], f32, tag="psq")
                for ps, xn, wn in ((psk, "k", "wk"), (psq, "q", "wq")):
                    for kc in range(KC):
                        nc.tensor.matmul(
                            ps[:, :L], x_ap(xs, xn, kc, j), w_ap(wn, kc, c),
                            start=(kc == 0), stop=(kc == KC - 1),
                        )
                # DVE may read at most one PSUM operand: stage K via SBUF
                k_sb = wpool.tile([128, EMB], f32, tag="k_sb")
                nc.scalar.copy(k_sb[:, :L], psk[:, :L])
                a = wpool.tile([128, EMB], f32, tag="a")
                nc.vector.tensor_mul(a[:, :L], psq[:, :L], k_sb[:, :L])
                if kind == "mix":
                    moff = s["moff"]
                    am = wpool.tile([128, EMB], f32, tag="am")
                    nc.vector.scalar_tensor_tensor(
                        am[:, :L], pm_sb[:, moff:moff + L], -10000.0,
                        a[:, :L], op0=OP.mult, op1=OP.add,
                    )
                    a = am
                mneg = spool.tile([128, H], f32, tag="mneg")
                av = a[:, :L].rearrange("p (g j) -> p g j", j=c)
                nc.vector.tensor_reduce(mneg[:], av, axis=AX.X, op=OP.max,
                                        negate=True)
                t_m = wpool.tile([128, EMB], f32, tag="t_m")
                mneg_b = (mneg[:].rearrange("p (g o) -> p g o", o=1)
                          .broadcast_to((128, H, c)))
                nc.gpsimd.tensor_add(
                    t_m[:, :L].rearrange("p (g j) -> p g j", j=c), av, mneg_b)
                e = lpool.tile([128, EMB], bf16, tag="e")
                # Q_len row mask rides the exp bias: dead rows get -1e4 so
                # e == 0 there (the resulting 0*inf NaNs in dead rows are
                # zeroed by the host scatter).
                nc.scalar.activation(e[:, :L], t_m[:, :L], ACTF.Exp,
                                     bias=qm_sb[:, j:j + 1])
                return e, psv

            def back(j, e, psv):
                s = slots[j]
                c, L = s["c"], s["L"]
                ev = e[:, :L].rearrange("p (g j) -> p g j", j=c)
                ssum = spool.tile([128, H], f32, tag="ssum")
                nc.vector.tensor_reduce(ssum[:], ev, axis=AX.X, op=OP.add)
                r = spool.tile([128, H], bf16, tag="r")
                with nc.allow_low_precision(reason="1/S at bf16: ~0.4% on softmax weights, well under the 2e-2 gate"):
                    nc.vector.reciprocal(r[:], ssum[:])
                p = wpool.tile([128, EMB], bf16, tag="p")
                r_b = (r[:].rearrange("p (g o) -> p g o", o=1)
                       .broadcast_to((128, H, c)))
                nc.gpsimd.tensor_mul(
                    p[:, :L].rearrange("p (g j) -> p g j", j=c), ev, r_b)
                nc.vector.tensor_mul(o_slice(j), p[:, :L], psv[:, :L])

            done = set()

            def flush(j):
                done.add(j)
                sp = j // SUP
                if sp in otiles and all(k in done
                                        for k in range(sup_lo[sp], sup_hi[sp])):
                    store_sup(sp)
                    del otiles[sp]

            xs_cur = xs0
            pending = None
            for j in range(nslot + 1):
                if j < nslot:
                    sup, local = divmod(j, SUP)
                    if local == 0 and sup > 0:
                        xs_cur = load_sup(sup)
                    res = front(j, xs_cur)
                else:
                    res = None
                if pending is not None:
                    back(pending[0], *pending[1])
                    flush(pending[0])
                    pending = None
                if j < nslot:
                    if res is None:
                        flush(j)
                    else:
                        pending = (j, res)

    nc.finalize()
    return nc


def _prep_inputs(Q_seq, K_seq, V_seq, Q_len, V_len, WQ, WK, WV):
    slots, assign, total_L, mix_L = _plan(Q_len, V_len)
    f16 = np.float16
    bf = ml_dtypes.bfloat16
    nslot = len(slots)

    wq_h = np.ascontiguousarray((WQ * 0.125).astype(f16))
    wk_h = np.ascontiguousarray(WK.astype(f16))
    wv_h = np.ascontiguousarray(WV.astype(f16))

    need_qk = {ba for i in range(NCORES) for j, s in enumerate(slots)
               if s["kind"] != "vonly" and assign[i][j] is not None
               for ba in [assign[i][j][0]]}
    need_v = {ba for i in range(NCORES) for j in range(nslot)
              if assign[i][j] is not None
              for ba in [assign[i][j][0]]}
    qT = {b: np.ascontiguousarray(Q_seq[b].T.astype(f16)) for b in need_qk}
    kT = {b: np.ascontiguousarray(K_seq[b].T.astype(f16)) for b in need_qk}
    vT = {b: np.ascontiguousarray(V_seq[b].T.astype(f16)) for b in need_v}

    in_maps = []
    for i in range(NCORES):
        qg = np.zeros((EMB, nslot * 128), f16)
        kg = np.zeros((EMB, nslot * 128), f16)
        vg = np.zeros((EMB, nslot * 128), f16)
        qmv = np.zeros((128, nslot), np.float32)
        pmv = np.zeros((128, mix_L), bf) if mix_L else None
        for j, s in enumerate(slots):
            ent = assign[i][j]
            if ent is None:
                continue
            b, tok0 = ent
            cs = slice(j * 128, (j + 1) * 128)
            ts = slice(tok0, tok0 + 128)
            vg[:, cs] = vT[b][:, ts]
            ql = int(Q_len[b, 0])
            live = int(np.clip(ql - tok0, 0, 128))
            if s["kind"] == "vonly":
                # multiplicative scale on the V copy (folds the 1/64)
                qmv[:live, j] = 1.0 / 64
            else:
                # additive exp bias: -1e4 on dead rows zeroes e there
                qmv[live:, j] = -1e4
            if s["kind"] != "vonly":
                qg[:, cs] = qT[b][:, ts]
                kg[:, cs] = kT[b][:, ts]
            if s["kind"] == "mix":
                vl = int(V_len[b, 0])
                c = s["c"]
                if vl < c:
                    dead = np.zeros((H, c), np.float32)
                    dead[:, vl:] = 1.0
                    pmv[:, s["moff"]:s["moff"] + s["L"]] = \
                        np.broadcast_to(dead.reshape(-1), (128, s["L"]))
        m = {
            "qg": qg, "kg": kg, "vg": vg,
            "wq": wq_h, "wk": wk_h, "wv": wv_h,
            "qm": np.ascontiguousarray(qmv),
        }
        if mix_L:
            m["pm"] = np.ascontiguousarray(pmv)
        in_maps.append(m)
    return in_maps, slots, assign, total_L


def _run(inputs, trace=False, mm_dtype_name="", tmpdir=None):
    from concourse.bass_utils import run_bass_kernel_spmd

    Q_len = np.asarray(inputs["Q_len"])
    V_len = np.asarray(inputs["V_len"])
    in_maps, slots, assign, total_L = _prep_inputs(
        np.asarray(inputs["Q_seq"]), np.asarray(inputs["K_seq"]),
        np.asarray(inputs["V_seq"]), Q_len, V_len,
        np.asarray(inputs["WQ"]), np.asarray(inputs["WK"]),
        np.asarray(inputs["WV"]))

    key = tuple((s["kind"], s["L"]) for s in slots)
    if key not in _CACHE:
        mix_L = sum(s["L"] for s in slots if s["kind"] == "mix")
        _CACHE[key] = _build(slots, total_L, mix_L)
    nc = _CACHE[key]

    res = run_bass_kernel_spmd(nc, in_maps, core_ids=list(range(NCORES)),
                               trace=trace, tmpdir=tmpdir)

    out = np.zeros((B, S, H * D), np.float32)
    for i in range(NCORES):
        po = res.results[i]["outp"].astype(np.float32)
        for j, s in enumerate(slots):
            ent = assign[i][j]
            if ent is None:
                continue
            b, tok0 = ent
            c, L, off = s["c"], s["L"], s["off"]
            live = int(np.clip(int(Q_len[b, 0]) - tok0, 0, 128))
            block = po[:live, off:off + L].reshape(live, H, c)
            if s["kind"] == "vonly":
                out[b, tok0:tok0 + live] = block.reshape(live, H * D)
            else:
                vl = int(V_len[b, 0])
                out[b, tok0:tok0 + live].reshape(live, H, D)[:, :, :vl] = \
                    block[:, :, :vl]
    return out, res


def kernel(Q_seq, K_seq, V_seq, Q_len, V_len, WQ, WK, WV):
    out, _ = _run(dict(Q_seq=Q_seq, K_seq=K_seq, V_seq=V_seq,
                       Q_len=Q_len, V_len=V_len, WQ=WQ, WK=WK, WV=WV))
    return out
